# revision 17
# baseline (speedup 1.0000x reference)
"""Multi-head attention (B=4, S=2048, D=1024, H=16) on 8 Trainium2 cores.

Sharding: (batch, head-group) grid — core c handles batch c//2, heads
(c%2)*8..(c%2)*8+8. Zero duplicated FLOPs; host sums the two partial
out-projections per batch and adds bo.

Per-core kernel, fp16 matmul operands (fp32 PSUM accumulation).

Layouts (all on-chip, no transposes anywhere):
  K^T/Q^T [512, 2048] feature-major; V [tok, head, 65] token-major with a
  ones column per head; scores computed k-major: S^T[k,q] = K.Q^T, so the
  exp output IS P^T, and the ones column makes the PV matmul accumulate the
  softmax denominator in PSUM row 64.

v3 design notes (from v2's trace):
  * This toolchain loads PE weights with a single foreground buffer: each
    LDWEIGHTS occupies the array after the prior matmul's fill, costing
    ~75-100ns per matmul on top of the N=512 stream (213ns). Fixes:
      - Scores: the two heads of a pair contract over disjoint row halves
        (partitions 0-63 / 64-127). With a SHARED st PSUM tile (both
        heads' tiles released by their exps together) the scheduler keeps
        the pair adjacent, so LDW/stream of head1 overlaps head0.
      - K/Q projections run two q-slices per weight chunk and the
        out-projection runs both output halves per AON chunk, emitting
        back-to-back matmuls with IDENTICAL stationary operands; the
        dedupe_ldweights() post-pass rewrites the second LDWEIGHTS of
        such a pair into a NOP (the array already holds the weights).
  * exp split: head0 tiles on ScalarE (table exp), head1 tiles entirely on
    VectorE via fp16 Schraudolph: K^T is pre-scaled by 2^10*log2(e)/8 so
    st = 1024*log2e*(q.k/8), and int16(st + 15360-C) bit-viewed as fp16
    is 2^(st/1024 - C/1024). Keeping whole heads on one approximation
    makes the sawtooth cancel in that head's softmax normalization
    (measured ~3e-3 max output error vs 2e-2 budget). ScalarE's exp
    carries bias -C*ln2/1024 so both paths share the 2^(-C/1024) factor.
  * ScalarE also absorbs the K/Q bias-adds and the PSUM->SBUF copies of
    the out-projection (it sits closer to PSUM); VectorE keeps the V bias,
    AO copies, reciprocals and normalize-muls, plus all head1 exps.
  * Short denominator chain: reciprocal in place on the AO row, one DRAM
    bounce for the partition-broadcast, then one normalize mul.
"""

import numpy as np

import bass_rust
import concourse.bass as bass
import concourse.tile as tile
from concourse import mybir

F32 = mybir.dt.float32
I16 = mybir.dt.int16
MMD = mybir.dt.float16     # matmul operand dtype

B, S, D = 4, 2048, 1024
NH, DK = 16, 64            # total heads, head dim
HG = 8                     # heads per core (head group)
DHG = HG * DK              # 512 features per head group
NP = 4                     # pairs of heads per core
QS = 512                   # q-slice size
NQS = S // QS              # 4
KT = S // 128              # 16 k-tiles
CT = D // 128              # 8 contraction chunks for projections
VW = DK + 1                # 65: V columns per head incl. ones column

# --- Schraudolph fp16 exp constants ------------------------------------
KSCALE = 1024.0 * np.log2(np.e) / 8.0          # K^T pre-scale: 184.66496...
C_SCH = 36.0                                   # sawtooth centering constant
B_SCH = 15360.0 - C_SCH                        # DVE add immediate
ACT_SCALE = float(np.log(2.0) / 1024.0)        # st -> s
ACT_BIAS = float(-C_SCH * np.log(2.0) / 1024.0)  # match DVE's 2^(-C/1024)


def split_multi_waits(nc):
    """This toolchain's walrus accepts only ONE sync-wait per instruction;
    Tile attaches several (one per producer proc). Hoist all but one wait
    onto single-wait NOPs inserted just before the instruction on the same
    engine (engines are in-order, so semantics are identical)."""
    uid = 0
    for f in nc.m.functions:
        for bb in f.blocks:
            il = bb.instructions
            i = 0
            while i < len(il):
                inst = il[i]
                si = inst.sync_info
                if si is not None and len(si.on_wait) > 1:
                    waits = list(si.on_wait)
                    inst.sync_info = bass_rust.SyncInfo(
                        on_wait=[waits[-1]], on_update=list(si.on_update)
                    )
                    for w in waits[:-1]:
                        nop = mybir.InstNoOp(
                            name=f"WSPLIT-{uid}",
                            engine=inst.engine,
                            bass_nofuse=True,
                            sync_info=bass_rust.SyncInfo(
                                on_wait=[w], on_update=[]
                            ),
                        )
                        uid += 1
                        il.insert(i, nop)
                        i += 1
                i += 1


def dedupe_ldweights(nc):
    """Rewrite an LDWEIGHTS whose stationary operand is identical to the
    immediately-preceding LDWEIGHTS into a NOP (keeping its syncs): the PE
    array already holds those weights, and skipping the reload removes the
    fill-blocking array load (~75-100ns/matmul). The matmul keeps listing
    the weights operand, so Tile's WAR protection on the buffer holds."""
    n = 0
    for f in nc.m.functions:
        for bb in f.blocks:
            il = bb.instructions
            prev_sig = None
            for i, inst in enumerate(il):
                if not isinstance(inst, mybir.InstLdweights):
                    continue
                a = inst.ins[0]
                sig = (
                    a.memref, a.offset, str(a.ap), str(a.dtype),
                    str(inst.tile_position), str(inst.perf_mode),
                    str(inst.is_transpose),
                )
                if sig == prev_sig:
                    il[i] = mybir.InstNoOp(
                        name=f"LDWDUP-{n}",
                        engine=inst.engine,
                        bass_nofuse=True,
                        sync_info=inst.sync_info,
                    )
                    n += 1
                else:
                    prev_sig = sig
    return n


def bcast_ap(ap, parts, n):
    """Partition-broadcast view of a DRAM row AP: [[0,parts],[1,n]]."""
    return bass.AP(tensor=ap.tensor, offset=ap.offset, ap=[[0, parts], [1, n]])


def build_kernel():
    nc = bass.Bass(trn_type="TRN2")

    xq = nc.dram_tensor("xq", (D, S), MMD, kind="ExternalInput")   # query[b].T
    xk = nc.dram_tensor("xk", (D, S), MMD, kind="ExternalInput")
    xv = nc.dram_tensor("xv", (D, S), MMD, kind="ExternalInput")
    wq = nc.dram_tensor("wq", (D, DHG), MMD, kind="ExternalInput")  # Wq[hg].T
    wk = nc.dram_tensor("wk", (D, DHG), MMD, kind="ExternalInput")  # *KSCALE
    wv = nc.dram_tensor("wv", (D, DHG), MMD, kind="ExternalInput")
    wo = nc.dram_tensor("wo", (DHG, D), MMD, kind="ExternalInput")  # Wo[:,hg].T
    bq = nc.dram_tensor("bq", (DHG,), F32, kind="ExternalInput")
    bk = nc.dram_tensor("bk", (DHG,), F32, kind="ExternalInput")    # *KSCALE
    bv = nc.dram_tensor("bv", (DHG,), F32, kind="ExternalInput")
    out = nc.dram_tensor("out", (S, D), MMD, kind="ExternalOutput")

    from contextlib import ExitStack

    with tile.TileContext(nc) as tc, ExitStack() as ctx:
        persist = ctx.enter_context(tc.tile_pool(name="persist", bufs=1))
        KT_sb = persist.tile([128, NP, S], MMD)        # K^T: pair p rows
        QT_sb = persist.tile([128, NP, S], MMD)        # Q^T
        V_sb = persist.tile([128, KT, HG, VW], MMD)    # V token-major + ones
        AON = persist.tile([128, NP, S], MMD)          # normalized AO^T
        wk_sb = persist.tile([128, CT, DHG], MMD)
        wq_sb = persist.tile([128, CT, DHG], MMD)
        wv_sb = persist.tile([128, CT, DHG], MMD)
        wo_sb = persist.tile([128, NP, D], MMD)
        bq_sb = persist.tile([128, NP], F32)
        bk_sb = persist.tile([128, NP], F32)
        bv_bc = persist.tile([128, DHG], F32)
        actb = persist.tile([128, 1], F32)             # exp bias column
        nc.vector.memset(actb[:], ACT_BIAS)

        nc.sync.dma_start(wk_sb[:], wk.rearrange("(c p) n -> p c n", p=128))
        nc.sync.dma_start(wq_sb[:], wq.rearrange("(c p) n -> p c n", p=128))
        nc.sync.dma_start(wv_sb[:], wv.rearrange("(c p) n -> p c n", p=128))
        nc.sync.dma_start(wo_sb[:], wo.rearrange("(c p) n -> p c n", p=128))
        with nc.allow_non_contiguous_dma(reason="tiny bias loads"):
            nc.sync.dma_start(bq_sb[:], bq.rearrange("(t p) -> p t", p=128))
            nc.sync.dma_start(bk_sb[:], bk.rearrange("(t p) -> p t", p=128))
        nc.sync.dma_start(bv_bc[:], bcast_ap(bv[:], 128, DHG))
        nc.vector.memset(V_sb[:, :, :, DK], 1.0)       # ones columns

        pmm = ctx.enter_context(tc.tile_pool(name="pmm", bufs=1, space="PSUM"))
        xpool = ctx.enter_context(tc.tile_pool(name="xw", bufs=10))
        ptp = ctx.enter_context(tc.tile_pool(name="ptile", bufs=7))
        npool = ctx.enter_context(tc.tile_pool(name="norm", bufs=4))
        opool = ctx.enter_context(tc.tile_pool(name="ostage", bufs=2))
        dpool = ctx.enter_context(
            tc.tile_pool(name="dscratch", bufs=3, space="DRAM")
        )

        def window(xdram, qs, tag="xw", bufs=10):
            """Load the 8 contraction chunks of one 512-token slice."""
            win = []
            for ct in range(CT):
                xc = xpool.tile([128, QS], MMD, tag=tag, name="xw", bufs=bufs)
                nc.sync.dma_start(
                    xc[:], xdram[ct * 128:(ct + 1) * 128,
                                 qs * QS:(qs + 1) * QS]
                )
                win.append(xc)
            return win

        def kq_pair(xdram, w_sb, dst, b_sb, jt, qsa, qsb_, wina, winb):
            """Two q-slices through one weight pair-column; back-to-back
            matmuls share the stationary operand (deduped LDWEIGHTS)."""
            psa = pmm.tile([128, QS], F32, tag="pj", name="pja", bufs=2)
            psb = pmm.tile([128, QS], F32, tag="pj", name="pjb", bufs=2)
            for ct in range(CT):
                wap = w_sb[:, ct, jt * 128:(jt + 1) * 128]
                nc.tensor.matmul(psa[:], wap, wina[ct][:],
                                 start=(ct == 0), stop=(ct == CT - 1))
                nc.tensor.matmul(psb[:], wap, winb[ct][:],
                                 start=(ct == 0), stop=(ct == CT - 1))
            for qs, ps in ((qsa, psa), (qsb_, psb)):
                nc.scalar.add(
                    dst[:, jt, qs * QS:(qs + 1) * QS], ps[:], b_sb[:, jt:jt + 1]
                )

        def v_tiles(qs):
            """V_sb tok-tiles for one 512-token slice (4 tiles)."""
            win = window(xv, qs)
            for i in range(4):
                tt = qs * 4 + i
                ps = pmm.tile([128, DHG], F32, tag="pj", name="pjv", bufs=2)
                for ct in range(CT):
                    nc.tensor.matmul(
                        ps[:],
                        win[ct][:, i * 128:(i + 1) * 128],
                        wv_sb[:, ct, :],
                        start=(ct == 0), stop=(ct == CT - 1),
                    )
                nc.vector.tensor_add(
                    V_sb[:, tt, :, 0:DK],
                    ps[:].rearrange("p (h d) -> p h d", d=DK),
                    bv_bc[:].rearrange("p (h d) -> p h d", d=DK),
                )

        def av_group(p, g, ao, pts):
            """PV accumulation for k-tile pair g (both heads)."""
            for h2 in range(2):
                hh = 2 * p + h2
                for j in range(2):
                    kt = 2 * g + j
                    nc.tensor.matmul(
                        ao[h2][:],
                        V_sb[:, kt, hh, :],
                        pts[h2][:, j, :],
                        start=(kt == 0), stop=(kt == KT - 1),
                    )

        def attention(p, qsb, last=False):
            """One head-pair over one 512-wide q-slice."""
            q0 = qsb * QS
            ao = [
                pmm.tile([VW, QS], F32, tag=f"ao{h2}", name=f"ao{h2}")
                for h2 in range(2)
            ]
            prev = None
            for g in range(KT // 2):
                # one st tile PER HEAD: a shared tile serializes its two
                # readers (Tile sequences cross-engine readers of one PSUM
                # tile), which would put both exps on one critical chain.
                st = [
                    pmm.tile([128, 2, QS], F32, tag=f"st{h2}", name=f"st{h2}",
                             bufs=1)
                    for h2 in range(2)
                ]
                # interleaved emission: consecutive matmuls use disjoint
                # row halves of the PE array and overlap
                for j in range(2):
                    kt = 2 * g + j
                    for h2 in range(2):
                        lo = h2 * DK
                        nc.tensor.matmul(
                            st[h2][:, j, :],
                            KT_sb[lo:lo + DK, p, kt * 128:(kt + 1) * 128],
                            QT_sb[lo:lo + DK, p, q0:q0 + QS],
                            start=True, stop=True,
                        )
                # software pipeline: previous group's AV behind these scores
                if prev is not None:
                    av_group(p, g - 1, ao, prev)
                pt0 = ptp.tile([128, 2, QS], MMD, tag="pt0", name="pt0")
                nc.scalar.activation(
                    pt0[:], st[0][:],
                    mybir.ActivationFunctionType.Exp,
                    scale=ACT_SCALE, bias=actb[:],
                )
                pt1i = ptp.tile([128, 2, QS], I16, tag="pt1", name="pt1")
                nc.vector.tensor_scalar_add(pt1i[:], st[1][:], B_SCH)
                prev = (pt0, pt1i.bitcast(MMD))
            av_group(p, KT // 2 - 1, ao, prev)

            # epilogue: AO -> SBUF (frees PSUM), reciprocal of the two
            # denominator rows via a [2,512] -> [128,2,4] DRAM reshape (the
            # reshape spreads the rows across all lanes: DVE reciprocal is
            # an 8-cycle-per-lane-element iterative op), broadcast back to
            # 64 partitions, then normalize into AON.
            aos = []
            for h2 in range(2):
                a = npool.tile([VW, QS], MMD, tag=f"aos{h2}", name="aos")
                nc.scalar.copy(a[:], ao[h2][:])
                aos.append(a)
            # the whole denominator chain runs at background priority (its
            # consumer, the out-projection, trails by a full q-slice) and the
            # final multiply runs on the otherwise-idle GPSIMD engine -- so
            # none of it can head-of-line block the exps in the DVE queue.
            with tc.high_priority(offset=(-(10 ** 6) if not last else 0)):
                dn = dpool.tile([2, QS], MMD, tag="dn", name="dn")
                rcd = dpool.tile([2, QS], MMD, tag="rcd", name="rcd")
                with nc.allow_non_contiguous_dma(reason="denominator reshape"):
                    for h2 in range(2):
                        nc.sync.dma_start(dn[h2:h2 + 1, :], aos[h2][DK:VW, :])
                    rc = npool.tile([128, 2, 4], MMD, tag="rc", name="rc")
                    nc.sync.dma_start(
                        rc[:], dn[:].rearrange("h (p j) -> p h j", j=4)
                    )
                    with nc.allow_low_precision(reason="fp16 recip of den"):
                        nc.vector.reciprocal(rc[:], rc[:])
                    nc.sync.dma_start(
                        rcd[:].rearrange("h (p j) -> p h j", j=4), rc[:]
                    )
                for h2 in range(2):
                    rb = npool.tile([DK, QS], MMD, tag="rb", name="rb")
                    nc.sync.dma_start(rb[:], bcast_ap(rcd[h2], DK, QS))
                    nc.gpsimd.tensor_mul(
                        AON[h2 * DK:(h2 + 1) * DK, p, q0:q0 + QS],
                        aos[h2][0:DK, :],
                        rb[:],
                    )

        def outproj_tile(qsb, tt):
            """Out-projection for token tile tt (128 rows) of q-slice qsb.
            Both 512-wide output halves per AON chunk -> shared LDWEIGHTS."""
            q0 = qsb * QS
            ot = opool.tile([128, D], MMD, tag="ot", name="ot")
            po = [
                pmm.tile([128, 512], F32, tag="pj", name=f"po{oh}", bufs=2)
                for oh in range(2)
            ]
            for ci in range(NP):
                aap = AON[:, ci, q0 + tt * 128:q0 + (tt + 1) * 128]
                for oh in range(2):
                    nc.tensor.matmul(
                        po[oh][:], aap,
                        wo_sb[:, ci, oh * 512:(oh + 1) * 512],
                        start=(ci == 0), stop=(ci == NP - 1),
                    )
            for oh in range(2):
                nc.scalar.copy(ot[:, oh * 512:(oh + 1) * 512], po[oh][:])
            nc.sync.dma_start(out[q0 + tt * 128:q0 + (tt + 1) * 128, :], ot[:])

        # ---- emission schedule ---------------------------------------------
        # Normal priority: pair-0 prerequisites, the attention stream, and
        # out-projections. Everything else (V + remaining K/Q projections)
        # is demoted to background priority: the scheduler pulls it early
        # only when a data dependency demands it, and otherwise uses it to
        # fill PE idle slots -- keeping the PE dense and the clock warm.
        for qsa, qsb_ in ((0, 1), (2, 3)):
            wa = window(xk, qsa, tag="xwp", bufs=32)
            wb = window(xk, qsb_, tag="xwp", bufs=32)
            kq_pair(xk, wk_sb, KT_sb, bk_sb, 0, qsa, qsb_, wa, wb)
        wa = window(xq, 0, tag="xwp", bufs=32)
        wb = window(xq, 1, tag="xwp", bufs=32)
        kq_pair(xq, wq_sb, QT_sb, bq_sb, 0, 0, 1, wa, wb)

        with tc.high_priority(offset=-(10 ** 6)):
            for qs in range(NQS):
                v_tiles(qs)
            for qsa, qsb_ in ((0, 1), (2, 3)):
                wa = window(xk, qsa, tag="xwb", bufs=16)
                wb = window(xk, qsb_, tag="xwb", bufs=16)
                for jt in range(1, NP):
                    kq_pair(xk, wk_sb, KT_sb, bk_sb, jt, qsa, qsb_, wa, wb)
            for qsa, qsb_ in ((0, 1), (2, 3)):
                wa = window(xq, qsa, tag="xwb", bufs=16)
                wb = window(xq, qsb_, tag="xwb", bufs=16)
                for jt in (range(1, NP) if qsa == 0 else range(NP)):
                    kq_pair(xq, wq_sb, QT_sb, bq_sb, jt, qsa, qsb_, wa, wb)

        for qsb in range(NQS):
            for p in range(NP):
                attention(p, qsb, last=(qsb == NQS - 1))
                if qsb > 0:
                    outproj_tile(qsb - 1, p)
        for tt in range(NQS):
            outproj_tile(NQS - 1, tt)

    ndup = dedupe_ldweights(nc)
    split_multi_waits(nc)
    return nc


def _prep_inputs(query, key, value, Wq, bq, Wk, bk, Wv, bv, Wo, bo):
    """Build the 8 per-core input maps."""
    def cvt(a):
        return np.ascontiguousarray(a.astype(np.float16))

    xt = {}
    for nm, x in (("xq", query), ("xk", key), ("xv", value)):
        xt[nm] = [cvt(x[b].T) for b in range(B)]
    in_maps = []
    for c in range(8):
        b, g = divmod(c, 2)
        rows = slice(g * DHG, (g + 1) * DHG)
        in_maps.append({
            "xq": xt["xq"][b], "xk": xt["xk"][b], "xv": xt["xv"][b],
            "wq": cvt(Wq[rows, :].T),
            "wk": cvt(Wk[rows, :].T * KSCALE),
            "wv": cvt(Wv[rows, :].T),
            "wo": cvt(Wo[:, rows].T),
            "bq": np.ascontiguousarray(bq[rows]),
            "bk": np.ascontiguousarray(bk[rows] * KSCALE, dtype=np.float32),
            "bv": np.ascontiguousarray(bv[rows]),
        })
    return in_maps


_NC_CACHE = None


def run(inputs, trace=False):
    """Returns (full_output, BassKernelResults)."""
    global _NC_CACHE
    from concourse.bass_utils import run_bass_kernel_spmd

    inputs = {k: np.asarray(v, np.float32) for k, v in inputs.items()}
    in_maps = _prep_inputs(**inputs)
    if _NC_CACHE is None:
        _NC_CACHE = build_kernel()
    res = run_bass_kernel_spmd(
        _NC_CACHE, in_maps, core_ids=list(range(8)), trace=trace
    )
    bo = inputs["bo"]
    full = np.empty((B, S, D), np.float32)
    for b in range(B):
        full[b] = (
            res.results[2 * b]["out"].astype(np.float32)
            + res.results[2 * b + 1]["out"].astype(np.float32)
            + bo
        )
    return full, res


def kernel(**inputs):
    return run(inputs, trace=False)[0]
